# revision 1
# baseline (speedup 1.0000x reference)
"""Correlation cost-volume kernel for Trainium2 (8 NeuronCores, SPMD).

Problem: x1, x2: (8, 128, 160, 160) f32
         out[b, k=(di,dj), i, j] = sum_c x1[b,c,i,j] * x2pad[b,c,i+di,j+dj] / 128^2
         with di,dj in [0,9) (pad=4 each side), out: (8, 81, 160, 160) f32.

Strategy (data-parallel over batch, one sample per core):
  The c=128 contraction maps onto the PE array as local Gram matrices.
  Inputs are host-transposed to column-major (C, j, i) so that matmul
  operands are single-free-dim contiguous slices (a walrus requirement):
  for each 8-row x 16-col pixel tile of x1 (128 pixels = M), one fp32
  matmul against the 24-col x 16-row window of padded x2 (N=384, K=c=128)
  computes all cross inner-products.  PSUM is scaled by 1/16384, copied
  to SBUF (DVE/ACT alternating), and for each 16-pixel-column lane group
  only the 9 needed window columns are DMA'd out (banded store, 144/81 =
  1.78x output bytes).  The residual per-pixel-row shear (a per-lane
  offset no engine can express) is a pure layout permutation done on the
  host during unshard via a zero-copy as_strided view.
"""

import sys

for _p in ("/opt/trn_rl_repo",):
    if _p not in sys.path:
        sys.path.insert(0, _p)

from contextlib import ExitStack

import numpy as np

import concourse.bacc as bacc
import concourse.tile as tile
from concourse import mybir
from concourse.bass_utils import run_bass_kernel_spmd

F32 = mybir.dt.float32

C = 128          # channels = matmul contraction K
H = W = 160
PAD = 4
HP = WP = H + 2 * PAD        # 168
KD = 9                       # shifts per axis (di, dj in [0, 9))
TB = 16                      # tile rows
TW = 8                       # tile cols  -> M = TB*TW = 128 pixels
RR = TB + KD - 1             # window rows = 24 (innermost, contiguous)
RC = TW + KD - 1             # window cols = 16
NRECT = RR * RC              # 384 = matmul N (<= 512 fp32 PSUM bank)
NB = H // TB                 # 20 bands
NT = W // TW                 # 10 tiles per band
GRP = 2                      # bands per store group
NSG = NB // GRP              # 5 store groups
SCALE = 1.0 / (C * C)

N_CORES = 8

_CACHE = {}


def _build_nc():
    nc = bacc.Bacc("TRN2", target_bir_lowering=False, debug=False,
                   num_devices=N_CORES)
    # host-pretiled x1: [c, band, tile, p] with p = ct*TB + rt (col-major
    # pixels); host-prebanded col-major x2: [c, band, j, band_row]
    x1tl = nc.dram_tensor("x1tl", [C, NB, NT, TB * TW], F32,
                          kind="ExternalInput").ap()
    x2bd = nc.dram_tensor("x2bd", [C, NB, WP, RR], F32,
                          kind="ExternalInput").ap()
    # Full-rect Gram output (contiguous stores; host trims + shears):
    # [storegroup, lane p=(ct*TB+rt), band-in-grp, tile, col', row']
    outb = nc.dram_tensor("outb", [NSG, TB * TW, GRP, NT, RC, RR], F32,
                          kind="ExternalOutput").ap()

    with tile.TileContext(nc) as tc, ExitStack() as ctx:
        x1_pool = ctx.enter_context(tc.tile_pool(name="x1b", bufs=3))
        x2_pool = ctx.enter_context(tc.tile_pool(name="x2b", bufs=2))
        st_pool = ctx.enter_context(tc.tile_pool(name="stage", bufs=2))
        ps_pool = ctx.enter_context(tc.tile_pool(name="psum", bufs=6,
                                                 space="PSUM"))
        for sg in range(NSG):
            stage = st_pool.tile([C, GRP, NT, NRECT], F32)
            for g in range(GRP):
                b = sg * GRP + g
                x1b = x1_pool.tile([C, NT, TB * TW], F32)
                nc.sync.dma_start(out=x1b[:], in_=x1tl[:, b])
                x2b = x2_pool.tile([C, WP, RR], F32)
                nc.sync.dma_start(out=x2b[:], in_=x2bd[:, b])

                for t in range(NT):
                    j0 = t * TW
                    ps = ps_pool.tile([128, NRECT], F32)
                    nc.tensor.matmul(
                        ps[:],
                        lhsT=x1b[:, t, :],
                        rhs=x2b[:, j0:j0 + RC, :],
                        start=True, stop=True,
                    )
                    # scaled PSUM -> SBUF copy, alternating engines
                    dst = stage[:, g, t, :]
                    if t % 2 == 0:
                        nc.vector.tensor_scalar_mul(dst, ps[:], SCALE)
                    else:
                        nc.scalar.mul(dst, ps[:], SCALE)

            # one fully-contiguous store per group: 128 descriptors of
            # GRP*NT*NRECT*4 bytes each, line-rate DMA
            nc.scalar.dma_start(out=outb[sg], in_=stage[:])
    nc.compile()
    return nc


def _get_nc():
    if "nc" not in _CACHE:
        _CACHE["nc"] = _build_nc()
    return _CACHE["nc"]


def _unshard_one(bnd: np.ndarray) -> np.ndarray:
    """(NSG, TB*TW, GRP, NT, RC, RR) full-rect Gram -> (81, H, W) output."""
    assert bnd.shape == (NSG, TB * TW, GRP, NT, RC, RR)
    b7 = bnd.reshape(NSG, TW, TB, GRP, NT, RC, RR)
    s = b7.strides
    # V[sg, ct, rt, g, t, dj, di] = b7[sg, ct, rt, g, t, ct + dj, rt + di]
    v = np.lib.stride_tricks.as_strided(
        b7, shape=(NSG, TW, TB, GRP, NT, KD, KD),
        strides=(s[0], s[1] + s[5], s[2] + s[6], s[3], s[4], s[5], s[6]))
    # out[di, dj, i=(sg,g,rt), j=(t,ct)]
    return np.ascontiguousarray(
        v.transpose(6, 5, 0, 3, 2, 4, 1).reshape(KD * KD, H, W))


def _install_ntff_hook():
    """The agent image's `antenv` lacks `axon_hooks`, so boot-time NTFF
    hook registration degraded silently.  Recreate the module and register
    the ctypes hook into libaxon_pjrt.so so trace=True produces profiles."""
    if "antenv.axon_hooks" in sys.modules:
        return
    import types

    mod = types.ModuleType("antenv.axon_hooks")
    state = {"hook": None}
    mod.set_axon_ntff_profile_hook = lambda h: state.__setitem__("hook", h)
    mod.get_axon_ntff_profile_hook = lambda: state["hook"]
    sys.modules["antenv.axon_hooks"] = mod
    import antenv

    antenv.axon_hooks = mod
    try:
        sys.path.insert(0, "/root/.axon_site")
        from trn_agent_boot.trn_boot import _ntff_profile_via_ctypes

        mod.set_axon_ntff_profile_hook(
            _ntff_profile_via_ctypes("/opt/axon/libaxon_pjrt.so"))
    except Exception as e:  # tracing degrades, run still works
        print(f"NTFF hook install failed: {e}", file=sys.stderr)


def _run(x1: np.ndarray, x2: np.ndarray, trace: bool = False):
    assert x1.shape == (N_CORES, C, H, W) and x2.shape == (N_CORES, C, H, W)
    x1 = np.asarray(x1, dtype=np.float32)
    # x1tl[n, c, b, t, ct*TB + rt] = x1[n, c, b*TB + rt, t*TW + ct]
    x1tl = np.ascontiguousarray(
        x1.reshape(N_CORES, C, NB, TB, NT, TW).transpose(0, 1, 2, 4, 5, 3)
        .reshape(N_CORES, C, NB, NT, TB * TW))
    x2p = np.pad(np.asarray(x2, dtype=np.float32),
                 ((0, 0), (0, 0), (PAD, PAD), (PAD, PAD)))
    # x2bd[n, c, b, j, r] = x2p[n, c, b*TB + r, j]  (bands overlap by 8 rows)
    s = x2p.strides
    x2bd = np.ascontiguousarray(np.lib.stride_tricks.as_strided(
        x2p, shape=(N_CORES, C, NB, WP, RR),
        strides=(s[0], s[1], TB * s[2], s[3], s[2])))
    if trace:
        _install_ntff_hook()
    nc = _get_nc()
    in_maps = [{"x1tl": x1tl[i], "x2bd": x2bd[i]} for i in range(N_CORES)]
    res = run_bass_kernel_spmd(nc, in_maps, list(range(N_CORES)), trace=trace)
    out = np.stack([_unshard_one(res.results[i]["outb"])
                    for i in range(N_CORES)])
    return out, res


def kernel(x1: np.ndarray, x2: np.ndarray) -> np.ndarray:
    out, _ = _run(np.asarray(x1), np.asarray(x2), trace=False)
    return out

